# revision 2
# baseline (speedup 1.0000x reference)
"""KV-cache scatter update kernel for Trainium2 (8 NeuronCores).

Problem: kv_cache (2L=4, B=8, H=8, S=4096, D=128) f32, new_kv (L=2, 2, B=8,
H=8, 1, D=128) f32, position_ids (B=8, 1) int. Output = kv_cache with
new_kv[l, kv, b, h, 0, :] written at [2l+kv, b, h, pos[b], :].

Sharding: split on the H dim (size 8) across the 8 cores. Each core:
  - copies its (4, 8, 4096, 128) cache slice DRAM->DRAM (67.1 MB)
  - scatters its 32 new rows (one per (layer', batch)) at runtime offsets
    computed on-device from position_ids via an indirect DMA.
"""

import numpy as np

import concourse.bacc as bacc
import concourse.bass as bass
import concourse.mybir as mybir
import concourse.tile as tile
from concourse.bass_utils import run_bass_kernel_spmd
from concourse.tile import add_dep_helper

L = 2
B = 8
H = 8
S = 4096
D = 128
NCORES = 8
LP = 2 * L          # 4 "layers" in the output (k/v interleaved)
ROWS = LP * B * S   # 131072 rows of D floats per core
NEW = LP * B        # 32 scattered rows per core

_NC_CACHE = {}


def _build(chain_k: int = 1):
    """Build the Bass module (one NEFF, same program on all 8 cores).

    chain_k > 1 builds the same per-iteration body (bulk copy + scatter)
    repeated and dependency-chained K times — used only by the timing
    harness to measure steady-state per-iteration HW time via the slope
    method (dispatch overhead cancels).
    """
    global _NC_CACHE
    if chain_k in _NC_CACHE:
        return _NC_CACHE[chain_k]

    nc = bacc.Bacc(
        "TRN2",
        target_bir_lowering=False,
        debug=False,
        num_devices=NCORES,
    )
    kv = nc.dram_tensor("kv", [ROWS, D], mybir.dt.float32, kind="ExternalInput")
    newkv = nc.dram_tensor("newkv", [NEW, D], mybir.dt.float32, kind="ExternalInput")
    pos = nc.dram_tensor("pos", [NEW, 1], mybir.dt.int32, kind="ExternalInput")
    out = nc.dram_tensor("out", [ROWS, D], mybir.dt.float32, kind="ExternalOutput")

    with tile.TileContext(nc) as tc:
        with tc.tile_pool(name="sb", bufs=1) as pool:
            newt = pool.tile([NEW, D], mybir.dt.float32)
            post = pool.tile([NEW, 1], mybir.dt.int32)
            bast = pool.tile([NEW, 1], mybir.dt.int32)
            idxt = pool.tile([NEW, 1], mybir.dt.int32)

            # Stage the 32 new rows and the (replicated x4) positions in SBUF.
            nc.gpsimd.dma_start(out=newt[:], in_=newkv[:])
            nc.gpsimd.dma_start(out=post[:], in_=pos[:])

            # idx[p] = p*S + pos[p % 8]  (row index into the flat [ROWS, D] view)
            nc.gpsimd.iota(bast[:], pattern=[[0, 1]], base=0, channel_multiplier=S)
            nc.vector.tensor_tensor(
                out=idxt[:], in0=bast[:], in1=post[:], op=mybir.AluOpType.add
            )

            prev = None
            for _ in range(chain_k):
                # Bulk cache copy, DRAM->DRAM (67.1 MB), one HWDGE DMA.
                # With all 8 cores active this runs at ~337 GB/s r+w HBM
                # traffic per core (~94% of the 358 GB/s per-NC limit).
                big = nc.sync.dma_start(out=out[:], in_=kv[:])
                if prev is not None:
                    add_dep_helper(big.ins, prev.ins, reason="chain iterations")

                # Scatter the 32 new rows over the copy at runtime offsets.
                sc = nc.gpsimd.indirect_dma_start(
                    out=out[:],
                    out_offset=bass.IndirectOffsetOnAxis(ap=idxt[:, :1], axis=0),
                    in_=newt[:],
                    in_offset=None,
                )
                add_dep_helper(
                    sc.ins, big.ins, reason="scatter must land after bulk copy"
                )
                prev = sc

    nc.compile()
    _NC_CACHE[chain_k] = nc
    return nc


def kernel(kv_cache, new_kv, position_ids):
    kv_cache = np.asarray(kv_cache)
    new_kv = np.asarray(new_kv)
    position_ids = np.asarray(position_ids)

    nc = _build()

    # positions replicated for all 4 (layer', ) groups: row p holds pos[p % 8]
    pos32 = np.tile(position_ids[:, 0].astype(np.int32), LP).reshape(NEW, 1)
    pos32 = np.ascontiguousarray(pos32)

    in_maps = []
    for h in range(NCORES):
        kv_h = np.ascontiguousarray(kv_cache[:, :, h, :, :]).reshape(ROWS, D)
        new_h = np.ascontiguousarray(new_kv[:, :, :, h, 0, :]).reshape(NEW, D)
        in_maps.append({"kv": kv_h, "newkv": new_h, "pos": pos32})

    res = run_bass_kernel_spmd(nc, in_maps, core_ids=list(range(NCORES)))
    outs = [r["out"].reshape(LP, B, S, D) for r in res.results]
    return np.stack(outs, axis=2)



# revision 6
# speedup vs baseline: 1.0105x; 1.0105x over previous
"""KV-cache scatter update kernel for Trainium2 (8 NeuronCores).

Problem: kv_cache (2L=4, B=8, H=8, S=4096, D=128) f32, new_kv (L=2, 2, B=8,
H=8, 1, D=128) f32, position_ids (B=8, 1) int. Output = kv_cache with
new_kv[l, kv, b, h, 0, :] written at [2l+kv, b, h, pos[b], :].

Sharding: split on the H dim (size 8) across the 8 cores. Each core:
  - copies its (4, 8, 4096, 128) cache slice DRAM->DRAM (67.1 MB)
  - scatters its 32 new rows (one per (layer', batch)) at runtime offsets
    computed on-device from position_ids via an indirect DMA.
"""

import numpy as np

import concourse.bacc as bacc
import concourse.bass as bass
import concourse.mybir as mybir
import concourse.tile as tile
from concourse.bass_utils import run_bass_kernel_spmd
from concourse.tile import add_dep_helper

L = 2
B = 8
H = 8
S = 4096
D = 128
NCORES = 8
LP = 2 * L          # 4 "layers" in the output (k/v interleaved)
ROWS = LP * B * S   # 131072 rows of D floats per core
NEW = LP * B        # 32 scattered rows per core

_NC_CACHE = {}


def _build(chain_k: int = 1, n_chunks: int = 1):
    """Build the Bass module (one NEFF, same program on all 8 cores).

    chain_k > 1 builds the same per-iteration body (bulk copy + scatter)
    repeated and dependency-chained K times — used only by the timing
    harness to measure steady-state per-iteration HW time via the slope
    method (dispatch overhead cancels).
    """
    global _NC_CACHE
    key = (chain_k, n_chunks)
    if key in _NC_CACHE:
        return _NC_CACHE[key]

    nc = bacc.Bacc(
        "TRN2",
        target_bir_lowering=False,
        debug=False,
        num_devices=NCORES,
    )
    kv = nc.dram_tensor("kv", [ROWS, D], mybir.dt.float32, kind="ExternalInput")
    newkv = nc.dram_tensor("newkv", [NEW, D], mybir.dt.float32, kind="ExternalInput")
    pos = nc.dram_tensor("pos", [NEW, 1], mybir.dt.int32, kind="ExternalInput")
    out = nc.dram_tensor("out", [ROWS, D], mybir.dt.float32, kind="ExternalOutput")

    with tile.TileContext(nc) as tc:
        with tc.tile_pool(name="sb", bufs=1) as pool:
            newt = pool.tile([NEW, D], mybir.dt.float32)
            post = pool.tile([NEW, 1], mybir.dt.int32)
            bast = pool.tile([NEW, 1], mybir.dt.int32)
            idxt = pool.tile([NEW, 1], mybir.dt.int32)

            # Stage the 32 new rows and the (replicated x4) positions in SBUF.
            nc.gpsimd.dma_start(out=newt[:], in_=newkv[:])
            nc.gpsimd.dma_start(out=post[:], in_=pos[:])

            # idx[p] = p*S + pos[p % 8]  (row index into the flat [ROWS, D] view)
            nc.gpsimd.iota(bast[:], pattern=[[0, 1]], base=0, channel_multiplier=S)
            nc.vector.tensor_tensor(
                out=idxt[:], in0=bast[:], in1=post[:], op=mybir.AluOpType.add
            )

            prev = None
            for _ in range(chain_k):
                # Bulk cache copy, DRAM->DRAM (67.1 MB), HWDGE DMA(s).
                # With all 8 cores active this runs at ~337 GB/s r+w HBM
                # traffic per core (~94% of the 358 GB/s per-NC limit).
                bigs = []
                rc = ROWS // n_chunks
                for c in range(n_chunks):
                    sl = slice(c * rc, (c + 1) * rc)
                    eng = nc.sync if c % 2 == 0 else nc.scalar
                    bigs.append(eng.dma_start(out=out[sl, :], in_=kv[sl, :]))
                if prev is not None:
                    for b in bigs:
                        add_dep_helper(b.ins, prev.ins, reason="chain iterations")

                # Scatter the 32 new rows over the copy at runtime offsets.
                sc = nc.gpsimd.indirect_dma_start(
                    out=out[:],
                    out_offset=bass.IndirectOffsetOnAxis(ap=idxt[:, :1], axis=0),
                    in_=newt[:],
                    in_offset=None,
                )
                for b in bigs:
                    add_dep_helper(
                        sc.ins, b.ins, reason="scatter must land after bulk copy"
                    )
                prev = sc

    nc.compile()
    _NC_CACHE[chain_k] = nc
    return nc


def kernel(kv_cache, new_kv, position_ids):
    kv_cache = np.asarray(kv_cache)
    new_kv = np.asarray(new_kv)
    position_ids = np.asarray(position_ids)

    nc = _build(n_chunks=2)

    # positions replicated for all 4 (layer', ) groups: row p holds pos[p % 8]
    pos32 = np.tile(position_ids[:, 0].astype(np.int32), LP).reshape(NEW, 1)
    pos32 = np.ascontiguousarray(pos32)

    in_maps = []
    for h in range(NCORES):
        kv_h = np.ascontiguousarray(kv_cache[:, :, h, :, :]).reshape(ROWS, D)
        new_h = np.ascontiguousarray(new_kv[:, :, :, h, 0, :]).reshape(NEW, D)
        in_maps.append({"kv": kv_h, "newkv": new_h, "pos": pos32})

    res = run_bass_kernel_spmd(nc, in_maps, core_ids=list(range(NCORES)))
    outs = [r["out"].reshape(LP, B, S, D) for r in res.results]
    return np.stack(outs, axis=2)

